# revision 13
# baseline (speedup 1.0000x reference)
"""ChebNet (K=2) graph classifier on 8 Trainium2 NeuronCores.

Strategy (graph/data parallel, zero halo):
  - The 50 batched graphs are independent (edges never cross graphs), so
    graphs are assigned whole to cores (LPT on edge counts, 6-7 per core).
  - Per graph, nodes are relabeled by in-degree (descending).  With that
    ordering, a padded ELL edge layout makes every segment-sum a plain
    PE matmul against a constant identity matrix: gather 128 source rows
    (one per destination rank in a 128-wide window) with dma_gather, then
    matmul-transpose-accumulate the chunks into PSUM.  No scatter, no
    one-hot selector matrices, no atomicity concerns.
  - Per layer: featT --(-dinv scale, PE transpose)--> fp16 row matrix in
    HBM --dma_gather--> edge-major chunks --PE identity matmuls--> PSUM
    aggT --(dinv scale)--> Tx1T --PE dense (W split in two K-tiles)-->
    next layer.  Max-pool readout and the classifier run on-device too.
  - All per-core tensors are host-prepared (index relabeling, sorting,
    padding: structural graph preprocessing only; all feature compute is
    on-device).  One SPMD program runs on all 8 cores; per-core work is
    padded to identical instruction counts (idle cores chew zero rows).
"""

import sys

if "/opt/trn_rl_repo" not in sys.path:
    sys.path.insert(0, "/opt/trn_rl_repo")

import numpy as np

# ---------------------------------------------------------------- constants
N = 100_000
E = 1_600_000
B = 50
GSIZE = 2000
D = 128  # IN == HID == 128
NCOUT = 10
NCORES = 8
NG = 7  # graph slots per core (50 = 2*7 + 6*6)
GBUD = 48  # max ELL chunks per dma_gather group


def _default_cfg():
    nwin = (GSIZE + 127) // 128
    return dict(
        ng=NG,
        gsize=GSIZE,  # real nodes per graph
        nwin=nwin,  # 128-wide dst windows per graph
        gstride=nwin * 128,  # padded nodes per graph slot
        gbud=GBUD,
    )


# ---------------------------------------------------------------- host prep
def _preprocess(src, dst, cfg, n_nodes, n_graphs):
    """Structural preprocessing: graph->core assignment, per-graph degree
    ordering, padded ELL tables, gather index streams.  Returns per-core
    arrays plus the (uniform) KW table and gather grouping."""
    gsize, ng, nwin, gstride = cfg["gsize"], cfg["ng"], cfg["nwin"], cfg["gstride"]

    deg = np.bincount(dst, minlength=n_nodes)
    eg = np.bincount(dst // gsize, minlength=n_graphs)

    # LPT assignment of graphs to cores (at most ng slots per core)
    order = np.argsort(-eg, kind="stable")
    loads = np.zeros(NCORES, dtype=np.int64)
    slots = [[] for _ in range(NCORES)]
    for g in order:
        cands = [c for c in range(NCORES) if len(slots[c]) < ng]
        c = min(cands, key=lambda i: (loads[i], len(slots[i])))
        slots[c].append(int(g))
        loads[c] += eg[g]

    # per-graph degree-descending permutation + relabeled ELL tables
    # perm[rank] = original local node id; inv[orig] = rank
    perms = np.zeros((n_graphs, gsize), dtype=np.int64)
    invs = np.zeros((n_graphs, gsize), dtype=np.int64)
    for g in range(n_graphs):
        dg = deg[g * gsize : (g + 1) * gsize]
        p = np.argsort(-dg, kind="stable")
        perms[g] = p
        invs[g][p] = np.arange(gsize)

    # sorted-by-rank edge lists per graph
    g_of_e = dst // gsize
    dst_rank = invs[g_of_e, dst - g_of_e * gsize]
    src_rank = invs[g_of_e, src - g_of_e * gsize]
    # order edges by (graph, dst_rank)
    eorder = np.lexsort((dst_rank, g_of_e))
    ge, de, se = g_of_e[eorder], dst_rank[eorder], src_rank[eorder]
    gbounds = np.searchsorted(ge, np.arange(n_graphs + 1))

    # KW per window = max over graphs of max degree inside the window;
    # with degree-sorted ranks that's the degree at rank 128*w.
    deg_sorted = np.zeros((n_graphs, gstride), dtype=np.int64)
    for g in range(n_graphs):
        dg = deg[g * gsize : (g + 1) * gsize]
        deg_sorted[g, :gsize] = dg[perms[g]]
    kw = np.maximum(deg_sorted[:, :: 128][:, :nwin].max(axis=0), 1).astype(np.int64)

    # pack windows into gather groups with sum(KW) <= gbud (uniform)
    groups = []  # list of (w_lo, w_hi) half-open
    w = 0
    while w < nwin:
        w2, tot = w, 0
        while w2 < nwin and tot + kw[w2] <= cfg["gbud"]:
            tot += kw[w2]
            w2 += 1
        if w2 == w:  # single window exceeding budget: take it alone
            w2 = w + 1
        groups.append((w, w2))
        w = w2

    # Per-graph ELL gather index table (FS row ids, fp16 row matrix layout:
    # rank r -> FS row (r % 128) * nwin + r // 128; pad -> last row of the
    # zero-padded region).
    pad_rank = gstride - 1 if gstride > gsize else gsize - 1
    # NOTE: gstride > gsize must hold for a guaranteed zero pad row unless
    # degree-0 nodes exist; enforced by callers (gsize not multiple of 128).
    assert gstride > gsize, "need zero pad rows in FS"

    def fsrow(rank):
        return (rank % 128) * nwin + rank // 128

    slot_idx_cols = []  # per slot: int16 [128, cols] wrapped index stream
    for c in range(NCORES):
        per_slot = []
        for s in range(ng):
            if s < len(slots[c]):
                g = slots[c][s]
                e0, e1 = gbounds[g], gbounds[g + 1]
                dr, sr = de[e0:e1], se[e0:e1]
                # position of each edge within its dst's list
                # (edges already sorted by dst rank; stable within graph)
                counts = np.bincount(dr, minlength=gstride)
                starts = np.concatenate([[0], np.cumsum(counts)[:-1]])
                pos = np.arange(e1 - e0) - starts[dr]
            else:
                dr = np.zeros(0, dtype=np.int64)
                sr = dr
                pos = dr
            flat_parts = []
            for (w_lo, w_hi) in groups:
                tot = int(kw[w_lo:w_hi].sum())
                arr = np.full((tot * 128,), fsrow(pad_rank), dtype=np.int64)
                c0 = 0
                for wdx in range(w_lo, w_hi):
                    if len(dr):
                        m = (dr >= wdx * 128) & (dr < (wdx + 1) * 128) & (
                            pos < kw[wdx]
                        )
                        p = dr[m] - wdx * 128
                        cc = pos[m]
                        arr[(c0 + cc) * 128 + p] = fsrow(sr[m])
                    c0 += kw[wdx]
                flat_parts.append(arr)
            flat = np.concatenate(flat_parts)
            # wrap-16 layout: unwrapped[i] = tile[i % 16, i // 16]
            a16 = flat.reshape(-1, 16).T.astype(np.int16)  # [16, cols]
            per_slot.append(np.tile(a16, (8, 1)))  # [128, cols]
        slot_idx_cols.append(np.concatenate(per_slot, axis=1))

    gidx = np.stack(slot_idx_cols)  # [NCORES, 128, ng*sum(kw)*8]

    dinv = np.clip(deg.astype(np.float64), 1.0, None) ** -0.5
    dinv = dinv.astype(np.float32)

    return dict(
        slots=slots,
        perms=perms,
        kw=kw,
        groups=groups,
        gidx=gidx,
        dinv=dinv,
    )


# ---------------------------------------------------------------- program
def _build_program(cfg, kw, groups):
    from concourse import bacc, mybir, tile
    import concourse.bass as bass

    ng, nwin, gstride, gsize = cfg["ng"], cfg["nwin"], cfg["gstride"], cfg["gsize"]
    ngg = ng * gstride
    sumkw = int(kw.sum())
    totc = ng * sumkw * 8  # gather idx columns overall
    gbudmax = max(int(kw[a:b].sum()) for a, b in groups)
    f16 = mybir.dt.float16
    f32 = mybir.dt.float32
    AL = mybir.AluOpType

    nc = bacc.Bacc(None, target_bir_lowering=False)

    xt_in = nc.declare_dram_parameter("XT", [128, ngg], f16, isOutput=False)
    ndiv_in = nc.declare_dram_parameter("NDIV", [128, ng * nwin], f32, isOutput=False)
    diag_in = nc.declare_dram_parameter("DIAG", [ng * nwin * 128, 128], f16, isOutput=False)
    gidx_in = nc.declare_dram_parameter("GIDX", [128, totc], mybir.dt.int16, isOutput=False)
    w1a_in = nc.declare_dram_parameter("W1A", [128, 128], f16, isOutput=False)
    w1b_in = nc.declare_dram_parameter("W1B", [128, 128], f16, isOutput=False)
    w2a_in = nc.declare_dram_parameter("W2A", [128, 128], f16, isOutput=False)
    w2b_in = nc.declare_dram_parameter("W2B", [128, 128], f16, isOutput=False)
    b1_in = nc.declare_dram_parameter("B1", [128, 1], f32, isOutput=False)
    b2_in = nc.declare_dram_parameter("B2", [128, 1], f32, isOutput=False)
    wc_in = nc.declare_dram_parameter("WC", [128, NCOUT], f16, isOutput=False)
    bc_in = nc.declare_dram_parameter("BC", [1, NCOUT], f16, isOutput=False)
    ones_in = nc.declare_dram_parameter("ONES1", [1, ng], f16, isOutput=False)
    id_in = nc.declare_dram_parameter("IDENT", [128, 128], f16, isOutput=False)
    out_dram = nc.declare_dram_parameter("OUT", [ng, NCOUT], f32, isOutput=True)

    fs = [
        [nc.dram_tensor(f"FS{l}_{s}", [gstride, 128], f16) for s in range(ng)]
        for l in range(2)
    ]

    # dense N tiling of the real columns
    ntiles = []
    off = 0
    while off < gsize:
        ln = min(500, gsize - off)
        ntiles.append((off, ln))
        off += ln

    with tile.TileContext(nc) as tc:
        with (
            tc.tile_pool(name="const", bufs=1) as cpool,
            tc.tile_pool(name="big", bufs=1) as bigpool,
            tc.tile_pool(name="work", bufs=2) as wpool,
            tc.tile_pool(name="tx1p", bufs=2) as tx1pool,
            tc.tile_pool(name="fsstage", bufs=2) as fspool,
            tc.tile_pool(name="msgs", bufs=3) as mpool,
            tc.tile_pool(name="gix", bufs=3) as gpool,
            tc.tile_pool(name="diagp", bufs=3) as dgpool,
            tc.tile_pool(name="ptr", bufs=2, space="PSUM") as ptrpool,
            tc.tile_pool(name="pwin", bufs=2, space="PSUM") as pwinpool,
            tc.tile_pool(name="pd", bufs=2, space="PSUM") as pdpool,
            tc.tile_pool(name="po", bufs=1, space="PSUM") as popool,
        ):
            ident = cpool.tile([128, 128], f16, tag="ident")
            w1a = cpool.tile([128, 128], f16, tag="w1a")
            w1b = cpool.tile([128, 128], f16, tag="w1b")
            w2a = cpool.tile([128, 128], f16, tag="w2a")
            w2b = cpool.tile([128, 128], f16, tag="w2b")
            b1t = cpool.tile([128, 1], f32, tag="b1")
            b2t = cpool.tile([128, 1], f32, tag="b2")
            wct = cpool.tile([128, NCOUT], f16, tag="wc")
            bct = cpool.tile([1, NCOUT], f16, tag="bc")
            ones1 = cpool.tile([1, ng], f16, tag="ones1")
            ndiv = cpool.tile([128, ng * nwin], f32, tag="ndiv")
            hg = cpool.tile([128, ng], f16, tag="hg")
            outs = cpool.tile([ng, NCOUT], f32, tag="outs")
            xt = bigpool.tile([128, ngg], f16, tag="xt")
            h1t = bigpool.tile([128, ngg], f16, tag="h1t")

            nc.sync.dma_start(out=ident[:], in_=id_in[:])
            nc.sync.dma_start(out=w1a[:], in_=w1a_in[:])
            nc.sync.dma_start(out=w1b[:], in_=w1b_in[:])
            nc.sync.dma_start(out=w2a[:], in_=w2a_in[:])
            nc.sync.dma_start(out=w2b[:], in_=w2b_in[:])
            nc.sync.dma_start(out=b1t[:], in_=b1_in[:])
            nc.sync.dma_start(out=b2t[:], in_=b2_in[:])
            nc.sync.dma_start(out=wct[:], in_=wc_in[:])
            nc.sync.dma_start(out=bct[:], in_=bc_in[:])
            nc.sync.dma_start(out=ones1[:], in_=ones_in[:])
            nc.sync.dma_start(out=ndiv[:], in_=ndiv_in[:])
            nc.sync.dma_start(out=xt[:], in_=xt_in[:])

            for layer in range(2):
                srcT = xt if layer == 0 else h1t
                wa, wb = (w1a, w1b) if layer == 0 else (w2a, w2b)
                bt = b1t if layer == 0 else b2t

                # ---- fp16 scaled row matrices into HBM (gather source):
                # transpose 128-col chunks on PE, scale rows by -dinv (per
                # partition scalar) on evacuation.
                for s in range(ng):
                    base = s * gstride
                    stg = fspool.tile([128, nwin, 128], f16, tag="stg")
                    for t in range(nwin):
                        ptr = ptrpool.tile([128, 128], f32, tag="ptr")
                        nc.tensor.matmul(
                            ptr[:],
                            srcT[:, base + t * 128 : base + (t + 1) * 128],
                            ident[:],
                            start=True,
                            stop=True,
                        )
                        nc.vector.tensor_scalar(
                            stg[:, t, :],
                            ptr[:],
                            ndiv[:, s * nwin + t : s * nwin + t + 1],
                            None,
                            AL.mult,
                        )
                    nc.sync.dma_start(
                        out=fs[layer][s][:].rearrange("(p t) f -> p t f", t=nwin),
                        in_=stg[:],
                    )

                # ---- gather + aggregate + dense, per graph slot
                for s in range(ng):
                    base = s * gstride
                    tx1 = tx1pool.tile([128, gstride], f16, tag="tx1")
                    coff = s * sumkw * 8  # idx column offset for this slot
                    for (w_lo, w_hi) in groups:
                        nw = w_hi - w_lo
                        tot = int(kw[w_lo:w_hi].sum())
                        gix = gpool.tile([128, gbudmax * 8], mybir.dt.int16, tag="gix")
                        nc.sync.dma_start(
                            out=gix[:, : tot * 8],
                            in_=gidx_in[:, coff : coff + tot * 8],
                        )
                        dgt = dgpool.tile([128, nwin, 128], f16, tag="dgt")
                        r0 = (s * nwin + w_lo) * 128
                        nc.sync.dma_start(
                            out=dgt[:, :nw, :],
                            in_=diag_in[r0 : r0 + nw * 128, :].rearrange(
                                "(w p) f -> p w f", p=128
                            ),
                        )
                        msgs = mpool.tile([128, gbudmax, 128], f16, tag="msgs")
                        nc.gpsimd.dma_gather(
                            msgs[:, :tot, :],
                            fs[layer][s][:],
                            gix[:, : tot * 8],
                            num_idxs=tot * 128,
                            num_idxs_reg=tot * 128,
                            elem_size=128,
                            single_packet=False,
                        )
                        c0 = 0
                        for wdx in range(w_lo, w_hi):
                            pwin = pwinpool.tile([128, 128], f32, tag="pwin")
                            kwi = int(kw[wdx])
                            for c in range(kwi):
                                nc.tensor.matmul(
                                    pwin[:],
                                    msgs[:, c0 + c, :],
                                    dgt[:, wdx - w_lo, :],
                                    start=(c == 0),
                                    stop=(c == kwi - 1),
                                )
                            c0 += kwi
                            nc.vector.tensor_copy(
                                tx1[:, wdx * 128 : (wdx + 1) * 128], pwin[:]
                            )
                        coff += tot * 8

                    # dense: h = relu([Tx0, Tx1] @ W + b)
                    if layer == 1:
                        h2 = wpool.tile([128, gstride], f16, tag="h2")
                    for (noff, nlen) in ntiles:
                        pd = pdpool.tile([128, 512], f32, tag="pd")
                        nc.tensor.matmul(
                            pd[:, :nlen],
                            wa[:],
                            srcT[:, base + noff : base + noff + nlen],
                            start=True,
                            stop=False,
                        )
                        nc.tensor.matmul(
                            pd[:, :nlen],
                            wb[:],
                            tx1[:, noff : noff + nlen],
                            start=False,
                            stop=True,
                        )
                        dsttile = (
                            h1t[:, base + noff : base + noff + nlen]
                            if layer == 0
                            else h2[:, noff : noff + nlen]
                        )
                        nc.vector.tensor_scalar(
                            dsttile,
                            pd[:, :nlen],
                            bt[:],
                            0.0,
                            AL.add,
                            AL.max,
                        )
                    if layer == 0 and gstride > gsize:
                        nc.vector.memset(
                            h1t[:, base + gsize : base + gstride], 0.0
                        )
                    if layer == 1:
                        nc.vector.tensor_reduce(
                            hg[:, s : s + 1],
                            h2[:, :gsize],
                            mybir.AxisListType.X,
                            AL.max,
                        )

            # ---- readout: out = HG^T @ Wc + 1^T @ bc
            po = popool.tile([ng, NCOUT], f32, tag="po")
            nc.tensor.matmul(po[:], hg[:, :ng], wct[:], start=True, stop=False)
            nc.tensor.matmul(po[:], ones1[:], bct[:], start=False, stop=True)
            nc.vector.tensor_copy(outs[:], po[:])
            nc.sync.dma_start(out=out_dram[:], in_=outs[:])

    nc.compile()
    return nc


# ---------------------------------------------------------------- host glue
def _make_core_inputs(x, W1, b1, W2, b2, Wc, bc, pre, cfg):
    ng, gstride, gsize, nwin = cfg["ng"], cfg["gstride"], cfg["gsize"], cfg["nwin"]
    ngg = ng * gstride
    in_maps = []
    for c in range(NCORES):
        xt = np.zeros((128, ngg), dtype=np.float16)
        ndiv = np.zeros((128, ng * nwin), dtype=np.float32)
        diag = np.zeros((ng * nwin * 128, 128), dtype=np.float16)
        for s, g in enumerate(pre["slots"][c]):
            perm = pre["perms"][g]
            xg = x[g * gsize : (g + 1) * gsize][perm]  # [gsize, 128] degree-ranked
            xt[:, s * gstride : s * gstride + gsize] = xg.T.astype(np.float16)
            dv = np.ones(gstride, dtype=np.float32)
            dv[:gsize] = pre["dinv"][g * gsize : (g + 1) * gsize][perm]
            ndiv[:, s * nwin : (s + 1) * nwin] = -dv.reshape(nwin, 128).T
            blk = diag[s * nwin * 128 : (s + 1) * nwin * 128].reshape(nwin, 128, 128)
            for w in range(nwin):
                np.fill_diagonal(blk[w], dv[w * 128 : (w + 1) * 128].astype(np.float16))
        in_maps.append(
            dict(
                XT=xt,
                NDIV=ndiv,
                DIAG=diag,
                GIDX=pre["gidx"][c],
                W1A=np.ascontiguousarray(W1[:128]).astype(np.float16),
                W1B=np.ascontiguousarray(W1[128:]).astype(np.float16),
                W2A=np.ascontiguousarray(W2[:128]).astype(np.float16),
                W2B=np.ascontiguousarray(W2[128:]).astype(np.float16),
                B1=b1.reshape(128, 1).astype(np.float32),
                B2=b2.reshape(128, 1).astype(np.float32),
                WC=Wc.astype(np.float16),
                BC=bc.reshape(1, NCOUT).astype(np.float16),
                ONES1=np.ones((1, ng), dtype=np.float16),
                IDENT=np.eye(128, dtype=np.float16),
            )
        )
    return in_maps


_CACHE = {}


def kernel(x, W1, b1, W2, b2, Wc, bc, src, dst, graph_ids, _trace=False):
    from concourse.bass_utils import run_bass_kernel_spmd

    x = np.asarray(x, dtype=np.float32)
    src = np.asarray(src).astype(np.int64)
    dst = np.asarray(dst).astype(np.int64)
    cfg = _default_cfg()

    pre = _preprocess(src, dst, cfg, N, B)
    key = (tuple(pre["kw"].tolist()), tuple(pre["groups"]))
    if key not in _CACHE:
        _CACHE[key] = _build_program(cfg, pre["kw"], pre["groups"])
    nc = _CACHE[key]

    in_maps = _make_core_inputs(
        np.asarray(x, np.float32),
        np.asarray(W1, np.float32),
        np.asarray(b1, np.float32),
        np.asarray(W2, np.float32),
        np.asarray(b2, np.float32),
        np.asarray(Wc, np.float32),
        np.asarray(bc, np.float32),
        pre,
        cfg,
    )
    res = run_bass_kernel_spmd(nc, in_maps, list(range(NCORES)), trace=_trace)

    out = np.zeros((B, NCOUT), dtype=np.float32)
    for c in range(NCORES):
        oc = res.results[c]["OUT"]
        for s, g in enumerate(pre["slots"][c]):
            out[g] = oc[s]
    if _trace:
        kernel._last_exec_ns = res.exec_time_ns
    return out


# revision 15
# speedup vs baseline: 10.9938x; 10.9938x over previous
"""ChebNet (K=2) graph classifier on 8 Trainium2 NeuronCores.

Strategy (graph/data parallel, zero halo):
  - The 50 batched graphs are independent (edges never cross graphs), so
    graphs are assigned whole to cores (6-7 per core).  One SPMD program
    runs on all 8 cores; cores with fewer graphs chew zero blocks.
  - The normalized aggregation  Tx1 = -D^-1/2 A D^-1/2 feat  is computed
    as dense 128x512-blocked matmuls on the PE against per-graph
    adjacency blocks S[s, d] = -dinv[s] * dinv[d] * count(s->d), built
    host-side (structural preprocessing: adjacency + degrees only) and
    streamed from HBM as plain sequential DMAs.  At avg degree 16 the
    dense blocks carry the same HBM traffic as a per-edge gather
    (2000*2000*2B = 8MB vs 32k*256B = 8.2MB per graph) but need no
    descriptor generation (which measures ~7ns/edge on the Q7 SWDGE
    path and dominates any gather-based variant).
  - Everything else stays feature-major on-chip: per-graph feature
    chunks are PE-transposed into node-major stationary tiles, the two
    Chebyshev dense layers run as K=128-split matmuls, max-pool readout
    and the classifier run on-device.  fp16 operands, fp32 PSUM.
"""

import sys

if "/opt/trn_rl_repo" not in sys.path:
    sys.path.insert(0, "/opt/trn_rl_repo")

import numpy as np

# ---------------------------------------------------------------- constants
N = 100_000
E = 1_600_000
B = 50
GSIZE = 2000
D = 128  # IN == HID == 128
NCOUT = 10
NCORES = 8
NG = 7  # graph slots per core (50 = 2*7 + 6*6)
NSLAB = 512  # dst columns per aggregation matmul


def _default_cfg():
    nwin = (GSIZE + 127) // 128
    gstride = nwin * 128
    nslab = min(NSLAB, gstride)
    return dict(
        ng=NG,
        gsize=GSIZE,
        nwin=nwin,
        gstride=gstride,
        nslab=nslab,
        nquad=gstride // nslab,
    )


# ---------------------------------------------------------------- host prep
def _preprocess(src, dst, cfg, n_nodes, n_graphs):
    """Structural preprocessing: graph->core assignment and per-graph
    scaled dense adjacency blocks."""
    gsize, ng, nwin, gstride = cfg["gsize"], cfg["ng"], cfg["nwin"], cfg["gstride"]

    deg = np.bincount(dst, minlength=n_nodes)
    dinv = (np.clip(deg.astype(np.float64), 1.0, None) ** -0.5).astype(np.float32)

    slots = [[] for _ in range(NCORES)]
    for g in range(n_graphs):
        slots[g % NCORES].append(g)

    # per-graph scaled dense adjacency, fp16, [gstride, gstride]
    g_of_e = dst // gsize
    flat = (src - g_of_e * gsize) * np.int64(gstride) + (dst - g_of_e * gsize)
    sblks = []
    for g in range(n_graphs):
        m = g_of_e == g
        cnt = np.bincount(flat[m], minlength=gstride * gstride).astype(np.float32)
        S = cnt.reshape(gstride, gstride)
        dv = np.zeros(gstride, dtype=np.float32)
        dv[:gsize] = dinv[g * gsize : (g + 1) * gsize]
        S *= -dv[:, None]
        S *= dv[None, :]
        sblks.append(S.astype(np.float16))
    return dict(slots=slots, sblks=sblks)


# ---------------------------------------------------------------- program
def _build_program(cfg):
    from concourse import bacc, mybir, tile

    ng, nwin, gstride, gsize, nquad = (
        cfg["ng"],
        cfg["nwin"],
        cfg["gstride"],
        cfg["gsize"],
        cfg["nquad"],
    )
    nslab = cfg["nslab"]
    ngg = ng * gstride
    f16 = mybir.dt.float16
    f32 = mybir.dt.float32
    AL = mybir.AluOpType

    nc = bacc.Bacc(None, target_bir_lowering=False)

    xt_in = nc.declare_dram_parameter("XT", [128, ngg], f16, isOutput=False)
    # S blocks: [ng, nquad, nwin, 128, NSLAB] src-chunk-major per dst-slab
    sb_in = nc.declare_dram_parameter(
        "SBLK", [ng * nquad * nwin * 128, nslab], f16, isOutput=False
    )
    w1a_in = nc.declare_dram_parameter("W1A", [128, 128], f16, isOutput=False)
    w1b_in = nc.declare_dram_parameter("W1B", [128, 128], f16, isOutput=False)
    w2a_in = nc.declare_dram_parameter("W2A", [128, 128], f16, isOutput=False)
    w2b_in = nc.declare_dram_parameter("W2B", [128, 128], f16, isOutput=False)
    b1_in = nc.declare_dram_parameter("B1", [128, 1], f32, isOutput=False)
    b2_in = nc.declare_dram_parameter("B2", [128, 1], f32, isOutput=False)
    wc_in = nc.declare_dram_parameter("WC", [128, NCOUT], f16, isOutput=False)
    bc_in = nc.declare_dram_parameter("BC", [1, NCOUT], f16, isOutput=False)
    ones_in = nc.declare_dram_parameter("ONES1", [1, ng], f16, isOutput=False)
    id_in = nc.declare_dram_parameter("IDENT", [128, 128], f16, isOutput=False)
    out_dram = nc.declare_dram_parameter("OUT", [ng, NCOUT], f32, isOutput=True)

    # dense N tiling of the real columns
    ntiles = []
    off = 0
    while off < gsize:
        ln = min(500, gsize - off)
        ntiles.append((off, ln))
        off += ln

    with tile.TileContext(nc) as tc:
        with (
            tc.tile_pool(name="const", bufs=1) as cpool,
            tc.tile_pool(name="big", bufs=1) as bigpool,
            tc.tile_pool(name="work", bufs=2) as wpool,
            tc.tile_pool(name="tx1p", bufs=2) as tx1pool,
            tc.tile_pool(name="stgp", bufs=2) as stgpool,
            tc.tile_pool(name="sblkp", bufs=3) as sbpool,
            tc.tile_pool(name="ptr", bufs=2, space="PSUM") as ptrpool,
            tc.tile_pool(name="pwin", bufs=2, space="PSUM") as pwinpool,
            tc.tile_pool(name="pd", bufs=2, space="PSUM") as pdpool,
            tc.tile_pool(name="po", bufs=1, space="PSUM") as popool,
        ):
            ident = cpool.tile([128, 128], f16, tag="ident")
            w1a = cpool.tile([128, 128], f16, tag="w1a")
            w1b = cpool.tile([128, 128], f16, tag="w1b")
            w2a = cpool.tile([128, 128], f16, tag="w2a")
            w2b = cpool.tile([128, 128], f16, tag="w2b")
            b1t = cpool.tile([128, 1], f32, tag="b1")
            b2t = cpool.tile([128, 1], f32, tag="b2")
            wct = cpool.tile([128, NCOUT], f16, tag="wc")
            bct = cpool.tile([1, NCOUT], f16, tag="bc")
            ones1 = cpool.tile([1, ng], f16, tag="ones1")
            hg = cpool.tile([128, ng], f16, tag="hg")
            outs = cpool.tile([ng, NCOUT], f32, tag="outs")
            xt = bigpool.tile([128, ngg], f16, tag="xt")
            h1t = bigpool.tile([128, ngg], f16, tag="h1t")

            nc.sync.dma_start(out=ident[:], in_=id_in[:])
            nc.sync.dma_start(out=w1a[:], in_=w1a_in[:])
            nc.sync.dma_start(out=w1b[:], in_=w1b_in[:])
            nc.sync.dma_start(out=w2a[:], in_=w2a_in[:])
            nc.sync.dma_start(out=w2b[:], in_=w2b_in[:])
            nc.sync.dma_start(out=b1t[:], in_=b1_in[:])
            nc.sync.dma_start(out=b2t[:], in_=b2_in[:])
            nc.sync.dma_start(out=wct[:], in_=wc_in[:])
            nc.sync.dma_start(out=bct[:], in_=bc_in[:])
            nc.sync.dma_start(out=ones1[:], in_=ones_in[:])
            nc.sync.dma_start(out=xt[:], in_=xt_in[:])

            for layer in range(2):
                srcT = xt if layer == 0 else h1t
                wa, wb = (w1a, w1b) if layer == 0 else (w2a, w2b)
                bt = b1t if layer == 0 else b2t

                for s in range(ng):
                    base = s * gstride

                    # node-major stationary chunks: stg[:, t, :] = srcT chunk^T
                    stg = stgpool.tile([128, nwin, 128], f16, tag="stg")
                    for t in range(nwin):
                        ptr = ptrpool.tile([128, 128], f32, tag="ptr")
                        nc.tensor.matmul(
                            ptr[:],
                            srcT[:, base + t * 128 : base + (t + 1) * 128],
                            ident[:],
                            start=True,
                            stop=True,
                        )
                        nc.vector.tensor_copy(stg[:, t, :], ptr[:])

                    # aggregation: Tx1T[:, slab] = sum_t stg_t^T @ S[t, slab]
                    tx1 = tx1pool.tile([128, gstride], f16, tag="tx1")
                    for q in range(nquad):
                        sb = sbpool.tile([128, nwin, nslab], f16, tag="sb")
                        r0 = ((s * nquad + q) * nwin) * 128
                        nc.sync.dma_start(
                            out=sb[:],
                            in_=sb_in[r0 : r0 + nwin * 128, :].rearrange(
                                "(t p) d -> p t d", p=128
                            ),
                        )
                        pwin = pwinpool.tile([128, nslab], f32, tag="pwin")
                        for t in range(nwin):
                            nc.tensor.matmul(
                                pwin[:],
                                stg[:, t, :],
                                sb[:, t, :],
                                start=(t == 0),
                                stop=(t == nwin - 1),
                            )
                        nc.vector.tensor_copy(
                            tx1[:, q * nslab : (q + 1) * nslab], pwin[:]
                        )

                    # dense: h = relu([Tx0, Tx1] @ W + b)
                    if layer == 1:
                        h2 = wpool.tile([128, gstride], f16, tag="h2")
                    for (noff, nlen) in ntiles:
                        pd = pdpool.tile([128, 512], f32, tag="pd")
                        nc.tensor.matmul(
                            pd[:, :nlen],
                            wa[:],
                            srcT[:, base + noff : base + noff + nlen],
                            start=True,
                            stop=False,
                        )
                        nc.tensor.matmul(
                            pd[:, :nlen],
                            wb[:],
                            tx1[:, noff : noff + nlen],
                            start=False,
                            stop=True,
                        )
                        dsttile = (
                            h1t[:, base + noff : base + noff + nlen]
                            if layer == 0
                            else h2[:, noff : noff + nlen]
                        )
                        nc.vector.tensor_scalar(
                            dsttile,
                            pd[:, :nlen],
                            bt[:],
                            0.0,
                            AL.add,
                            AL.max,
                        )
                    if layer == 0 and gstride > gsize:
                        nc.vector.memset(h1t[:, base + gsize : base + gstride], 0.0)
                    if layer == 1:
                        nc.vector.tensor_reduce(
                            hg[:, s : s + 1],
                            h2[:, :gsize],
                            mybir.AxisListType.X,
                            AL.max,
                        )

            # ---- readout: out = HG^T @ Wc + 1^T @ bc
            po = popool.tile([ng, NCOUT], f32, tag="po")
            nc.tensor.matmul(po[:], hg[:, :ng], wct[:], start=True, stop=False)
            nc.tensor.matmul(po[:], ones1[:], bct[:], start=False, stop=True)
            nc.vector.tensor_copy(outs[:], po[:])
            nc.sync.dma_start(out=out_dram[:], in_=outs[:])

    nc.compile()
    return nc


# ---------------------------------------------------------------- host glue
def _make_core_inputs(x, W1, b1, W2, b2, Wc, bc, pre, cfg):
    ng, gstride, gsize, nwin, nquad = (
        cfg["ng"],
        cfg["gstride"],
        cfg["gsize"],
        cfg["nwin"],
        cfg["nquad"],
    )
    nslab = cfg["nslab"]
    ngg = ng * gstride
    in_maps = []
    for c in range(NCORES):
        xt = np.zeros((128, ngg), dtype=np.float16)
        sblk = np.zeros((ng * nquad * nwin * 128, nslab), dtype=np.float16)
        sv = sblk.reshape(ng, nquad, nwin, 128, nslab)
        for s, g in enumerate(pre["slots"][c]):
            xg = x[g * gsize : (g + 1) * gsize]  # [gsize, 128]
            xt[:, s * gstride : s * gstride + gsize] = xg.T.astype(np.float16)
            # S_g [gstride, gstride] -> [q, t, p, d]
            Sg = pre["sblks"][g].reshape(nwin, 128, nquad, nslab)
            sv[s] = Sg.transpose(2, 0, 1, 3)
        in_maps.append(
            dict(
                XT=xt,
                SBLK=sblk,
                W1A=np.ascontiguousarray(W1[:128]).astype(np.float16),
                W1B=np.ascontiguousarray(W1[128:]).astype(np.float16),
                W2A=np.ascontiguousarray(W2[:128]).astype(np.float16),
                W2B=np.ascontiguousarray(W2[128:]).astype(np.float16),
                B1=b1.reshape(128, 1).astype(np.float32),
                B2=b2.reshape(128, 1).astype(np.float32),
                WC=Wc.astype(np.float16),
                BC=bc.reshape(1, NCOUT).astype(np.float16),
                ONES1=np.ones((1, ng), dtype=np.float16),
                IDENT=np.eye(128, dtype=np.float16),
            )
        )
    return in_maps


_CACHE = {}


def kernel(x, W1, b1, W2, b2, Wc, bc, src, dst, graph_ids, _trace=False):
    from concourse.bass_utils import run_bass_kernel_spmd

    x = np.asarray(x, dtype=np.float32)
    src = np.asarray(src).astype(np.int64)
    dst = np.asarray(dst).astype(np.int64)
    cfg = _default_cfg()

    pre = _preprocess(src, dst, cfg, N, B)
    key = "prog"
    if key not in _CACHE:
        _CACHE[key] = _build_program(cfg)
    nc = _CACHE[key]

    in_maps = _make_core_inputs(
        x,
        np.asarray(W1, np.float32),
        np.asarray(b1, np.float32),
        np.asarray(W2, np.float32),
        np.asarray(b2, np.float32),
        np.asarray(Wc, np.float32),
        np.asarray(bc, np.float32),
        pre,
        cfg,
    )
    res = run_bass_kernel_spmd(nc, in_maps, list(range(NCORES)), trace=_trace)

    out = np.zeros((B, NCOUT), dtype=np.float32)
    for c in range(NCORES):
        oc = res.results[c]["OUT"]
        for s, g in enumerate(pre["slots"][c]):
            out[g] = oc[s]
    if _trace:
        kernel._last_exec_ns = res.exec_time_ns
    return out


# revision 17
# speedup vs baseline: 12.3212x; 1.1207x over previous
"""ChebNet (K=2) graph classifier on 8 Trainium2 NeuronCores.

Strategy (graph/data parallel, zero halo):
  - The 50 batched graphs are independent (edges never cross graphs), so
    graphs are assigned whole to cores (6-7 per core).  One SPMD program
    runs on all 8 cores; cores with fewer graphs chew zero blocks.
  - The normalized aggregation  Tx1 = -D^-1/2 A D^-1/2 feat  is computed
    as dense 128x512-blocked matmuls on the PE against per-graph
    adjacency blocks S[s, d] = -dinv[s] * dinv[d] * count(s->d), built
    host-side (structural preprocessing: adjacency + degrees only) and
    streamed from HBM as plain sequential DMAs.  At avg degree 16 the
    dense blocks carry the same HBM traffic as a per-edge gather
    (2000*2000*2B = 8MB vs 32k*256B = 8.2MB per graph) but need no
    descriptor generation (which measures ~7ns/edge on the Q7 SWDGE
    path and dominates any gather-based variant).
  - Everything else stays feature-major on-chip: per-graph feature
    chunks are PE-transposed into node-major stationary tiles, the two
    Chebyshev dense layers run as K=128-split matmuls, max-pool readout
    and the classifier run on-device.  fp16 operands, fp32 PSUM.
"""

import sys

if "/opt/trn_rl_repo" not in sys.path:
    sys.path.insert(0, "/opt/trn_rl_repo")

import numpy as np

# ---------------------------------------------------------------- constants
N = 100_000
E = 1_600_000
B = 50
GSIZE = 2000
D = 128  # IN == HID == 128
NCOUT = 10
NCORES = 8
NG = 7  # graph slots per core (50 = 2*7 + 6*6)
NSLAB = 512  # dst columns per aggregation matmul


def _default_cfg():
    nwin = (GSIZE + 127) // 128
    gstride = nwin * 128
    nslab = 500 if GSIZE % 500 == 0 else GSIZE
    return dict(
        ng=NG,
        gsize=GSIZE,
        nwin=nwin,
        gstride=gstride,
        nslab=nslab,
        nquad=GSIZE // nslab,
    )


# ---------------------------------------------------------------- host prep
def _preprocess(src, dst, cfg, n_nodes, n_graphs):
    """Structural preprocessing: graph->core assignment and per-graph
    scaled dense adjacency blocks."""
    gsize, ng, nwin, gstride = cfg["gsize"], cfg["ng"], cfg["nwin"], cfg["gstride"]

    deg = np.bincount(dst, minlength=n_nodes)
    dinv = (np.clip(deg.astype(np.float64), 1.0, None) ** -0.5).astype(np.float32)

    order = [0, 2, 1, 3, 4, 5, 6, 7]  # extra graphs land on cores 0 and 2
    slots = [[] for _ in range(NCORES)]
    for g in range(n_graphs):
        slots[order[g % NCORES]].append(g)

    # per-graph scaled dense adjacency, fp16, [gstride, gstride]
    g_of_e = dst // gsize
    flat = (src - g_of_e * gsize) * np.int64(gstride) + (dst - g_of_e * gsize)
    sblks = []
    for g in range(n_graphs):
        m = g_of_e == g
        cnt = np.bincount(flat[m], minlength=gstride * gstride).astype(np.float32)
        S = cnt.reshape(gstride, gstride)
        dv = np.zeros(gstride, dtype=np.float32)
        dv[:gsize] = dinv[g * gsize : (g + 1) * gsize]
        S *= -dv[:, None]
        S *= dv[None, :]
        sblks.append(S[:, :gsize].astype(np.float16))
    return dict(slots=slots, sblks=sblks)


# ---------------------------------------------------------------- program
def _build_program(cfg):
    from concourse import bacc, mybir, tile

    ng, nwin, gstride, gsize, nquad = (
        cfg["ng"],
        cfg["nwin"],
        cfg["gstride"],
        cfg["gsize"],
        cfg["nquad"],
    )
    nslab = cfg["nslab"]
    ngg = ng * gstride
    f16 = mybir.dt.float16
    f32 = mybir.dt.float32
    AL = mybir.AluOpType

    nc = bacc.Bacc(None, target_bir_lowering=False)

    xt_in = nc.declare_dram_parameter("XT", [128, ngg], f16, isOutput=False)
    # S blocks: [ng, nquad, nwin, 128, NSLAB] src-chunk-major per dst-slab
    sb_in = nc.declare_dram_parameter(
        "SBLK", [ng * nquad * nwin * 128, nslab], f16, isOutput=False
    )
    w1a_in = nc.declare_dram_parameter("W1A", [128, 128], f16, isOutput=False)
    w1b_in = nc.declare_dram_parameter("W1B", [128, 128], f16, isOutput=False)
    w2a_in = nc.declare_dram_parameter("W2A", [128, 128], f16, isOutput=False)
    w2b_in = nc.declare_dram_parameter("W2B", [128, 128], f16, isOutput=False)
    b1_in = nc.declare_dram_parameter("B1", [128, 1], f32, isOutput=False)
    b2_in = nc.declare_dram_parameter("B2", [128, 1], f32, isOutput=False)
    wc_in = nc.declare_dram_parameter("WC", [128, NCOUT], f16, isOutput=False)
    bc_in = nc.declare_dram_parameter("BC", [1, NCOUT], f16, isOutput=False)
    ones_in = nc.declare_dram_parameter("ONES1", [1, ng], f16, isOutput=False)
    id_in = nc.declare_dram_parameter("IDENT", [128, 128], f16, isOutput=False)
    out_dram = nc.declare_dram_parameter("OUT", [ng, NCOUT], f32, isOutput=True)

    # dense N tiling of the real columns
    ntiles = []
    off = 0
    while off < gsize:
        ln = min(500, gsize - off)
        ntiles.append((off, ln))
        off += ln

    with tile.TileContext(nc) as tc:
        with (
            tc.tile_pool(name="const", bufs=1) as cpool,
            tc.tile_pool(name="big", bufs=1) as bigpool,
            tc.tile_pool(name="work", bufs=2) as wpool,
            tc.tile_pool(name="tx1p", bufs=2) as tx1pool,
            tc.tile_pool(name="stgp", bufs=2) as stgpool,
            tc.tile_pool(name="sblkp", bufs=4) as sbpool,
            tc.tile_pool(name="ptr", bufs=2, space="PSUM") as ptrpool,
            tc.tile_pool(name="pwin", bufs=2, space="PSUM") as pwinpool,
            tc.tile_pool(name="pd", bufs=2, space="PSUM") as pdpool,
            tc.tile_pool(name="po", bufs=1, space="PSUM") as popool,
        ):
            ident = cpool.tile([128, 128], f16, tag="ident")
            w1a = cpool.tile([128, 128], f16, tag="w1a")
            w1b = cpool.tile([128, 128], f16, tag="w1b")
            w2a = cpool.tile([128, 128], f16, tag="w2a")
            w2b = cpool.tile([128, 128], f16, tag="w2b")
            b1t = cpool.tile([128, 1], f32, tag="b1")
            b2t = cpool.tile([128, 1], f32, tag="b2")
            wct = cpool.tile([128, NCOUT], f16, tag="wc")
            bct = cpool.tile([1, NCOUT], f16, tag="bc")
            ones1 = cpool.tile([1, ng], f16, tag="ones1")
            hg = cpool.tile([128, ng], f16, tag="hg")
            outs = cpool.tile([ng, NCOUT], f32, tag="outs")
            xt = bigpool.tile([128, ngg], f16, tag="xt")
            h1t = bigpool.tile([128, ngg], f16, tag="h1t")

            nc.sync.dma_start(out=ident[:], in_=id_in[:])
            nc.sync.dma_start(out=w1a[:], in_=w1a_in[:])
            nc.sync.dma_start(out=w1b[:], in_=w1b_in[:])
            nc.sync.dma_start(out=w2a[:], in_=w2a_in[:])
            nc.sync.dma_start(out=w2b[:], in_=w2b_in[:])
            nc.sync.dma_start(out=b1t[:], in_=b1_in[:])
            nc.sync.dma_start(out=b2t[:], in_=b2_in[:])
            nc.sync.dma_start(out=wct[:], in_=wc_in[:])
            nc.sync.dma_start(out=bct[:], in_=bc_in[:])
            nc.sync.dma_start(out=ones1[:], in_=ones_in[:])
            for s_ in range(ng):
                nc.sync.dma_start(
                    out=xt[:, s_ * gstride : (s_ + 1) * gstride],
                    in_=xt_in[:, s_ * gstride : (s_ + 1) * gstride],
                )

            for layer in range(2):
                srcT = xt if layer == 0 else h1t
                wa, wb = (w1a, w1b) if layer == 0 else (w2a, w2b)
                bt = b1t if layer == 0 else b2t

                for s in range(ng):
                    base = s * gstride

                    # node-major stationary chunks: stg[:, t, :] = srcT chunk^T
                    stg = stgpool.tile([128, nwin, 128], f16, tag="stg")
                    for t in range(nwin):
                        ptr = ptrpool.tile([128, 128], f32, tag="ptr")
                        nc.tensor.matmul(
                            ptr[:],
                            srcT[:, base + t * 128 : base + (t + 1) * 128],
                            ident[:],
                            start=True,
                            stop=True,
                        )
                        nc.vector.tensor_copy(stg[:, t, :], ptr[:])

                    # aggregation: Tx1T[:, slab] = sum_t stg_t^T @ S[t, slab]
                    tx1 = tx1pool.tile([128, gsize], f16, tag="tx1")
                    for q in range(nquad):
                        sb = sbpool.tile([128, nwin, nslab], f16, tag="sb")
                        r0 = ((s * nquad + q) * nwin) * 128
                        nc.sync.dma_start(
                            out=sb[:],
                            in_=sb_in[r0 : r0 + nwin * 128, :].rearrange(
                                "(p t) d -> p t d", t=nwin
                            ),
                        )
                        pwin = pwinpool.tile([128, nslab], f32, tag="pwin")
                        for t in range(nwin):
                            nc.tensor.matmul(
                                pwin[:],
                                stg[:, t, :],
                                sb[:, t, :],
                                start=(t == 0),
                                stop=(t == nwin - 1),
                            )
                        nc.vector.tensor_copy(
                            tx1[:, q * nslab : (q + 1) * nslab], pwin[:]
                        )

                    # dense: h = relu([Tx0, Tx1] @ W + b)
                    if layer == 1:
                        h2 = wpool.tile([128, gsize], f16, tag="h2")
                    for (noff, nlen) in ntiles:
                        pd = pdpool.tile([128, 512], f32, tag="pd")
                        nc.tensor.matmul(
                            pd[:, :nlen],
                            wa[:],
                            srcT[:, base + noff : base + noff + nlen],
                            start=True,
                            stop=False,
                        )
                        nc.tensor.matmul(
                            pd[:, :nlen],
                            wb[:],
                            tx1[:, noff : noff + nlen],
                            start=False,
                            stop=True,
                        )
                        dsttile = (
                            h1t[:, base + noff : base + noff + nlen]
                            if layer == 0
                            else h2[:, noff : noff + nlen]
                        )
                        nc.vector.tensor_scalar(
                            dsttile,
                            pd[:, :nlen],
                            bt[:],
                            0.0,
                            AL.add,
                            AL.max,
                        )
                    if layer == 0 and gstride > gsize:
                        nc.vector.memset(h1t[:, base + gsize : base + gstride], 0.0)
                    if layer == 1:
                        nc.vector.tensor_reduce(
                            hg[:, s : s + 1],
                            h2[:, :gsize],
                            mybir.AxisListType.X,
                            AL.max,
                        )

            # ---- readout: out = HG^T @ Wc + 1^T @ bc
            po = popool.tile([ng, NCOUT], f32, tag="po")
            nc.tensor.matmul(po[:], hg[:, :ng], wct[:], start=True, stop=False)
            nc.tensor.matmul(po[:], ones1[:], bct[:], start=False, stop=True)
            nc.vector.tensor_copy(outs[:], po[:])
            nc.sync.dma_start(out=out_dram[:], in_=outs[:])

    nc.compile()
    return nc


# ---------------------------------------------------------------- host glue
def _make_core_inputs(x, W1, b1, W2, b2, Wc, bc, pre, cfg):
    ng, gstride, gsize, nwin, nquad = (
        cfg["ng"],
        cfg["gstride"],
        cfg["gsize"],
        cfg["nwin"],
        cfg["nquad"],
    )
    nslab = cfg["nslab"]
    ngg = ng * gstride
    in_maps = []
    for c in range(NCORES):
        xt = np.zeros((128, ngg), dtype=np.float16)
        sblk = np.zeros((ng * nquad * nwin * 128, nslab), dtype=np.float16)
        sv = sblk.reshape(ng, nquad, 128, nwin, nslab)
        for s, g in enumerate(pre["slots"][c]):
            xg = x[g * gsize : (g + 1) * gsize]  # [gsize, 128]
            xt[:, s * gstride : s * gstride + gsize] = xg.T.astype(np.float16)
            # S_g [gstride, gsize] -> [q, p, t, d] (p-major rows: contiguous
            # 16-row reads per partition -> large DMA descriptors)
            Sg = pre["sblks"][g].reshape(nwin, 128, nquad, nslab)
            sv[s] = Sg.transpose(2, 1, 0, 3)
        in_maps.append(
            dict(
                XT=xt,
                SBLK=sblk,
                W1A=np.ascontiguousarray(W1[:128]).astype(np.float16),
                W1B=np.ascontiguousarray(W1[128:]).astype(np.float16),
                W2A=np.ascontiguousarray(W2[:128]).astype(np.float16),
                W2B=np.ascontiguousarray(W2[128:]).astype(np.float16),
                B1=b1.reshape(128, 1).astype(np.float32),
                B2=b2.reshape(128, 1).astype(np.float32),
                WC=Wc.astype(np.float16),
                BC=bc.reshape(1, NCOUT).astype(np.float16),
                ONES1=np.ones((1, ng), dtype=np.float16),
                IDENT=np.eye(128, dtype=np.float16),
            )
        )
    return in_maps


_CACHE = {}


def kernel(x, W1, b1, W2, b2, Wc, bc, src, dst, graph_ids, _trace=False):
    from concourse.bass_utils import run_bass_kernel_spmd

    x = np.asarray(x, dtype=np.float32)
    src = np.asarray(src).astype(np.int64)
    dst = np.asarray(dst).astype(np.int64)
    cfg = _default_cfg()

    pre = _preprocess(src, dst, cfg, N, B)
    key = "prog"
    if key not in _CACHE:
        _CACHE[key] = _build_program(cfg)
    nc = _CACHE[key]

    in_maps = _make_core_inputs(
        x,
        np.asarray(W1, np.float32),
        np.asarray(b1, np.float32),
        np.asarray(W2, np.float32),
        np.asarray(b2, np.float32),
        np.asarray(Wc, np.float32),
        np.asarray(bc, np.float32),
        pre,
        cfg,
    )
    res = run_bass_kernel_spmd(nc, in_maps, list(range(NCORES)), trace=_trace)

    out = np.zeros((B, NCOUT), dtype=np.float32)
    for c in range(NCORES):
        oc = res.results[c]["OUT"]
        for s, g in enumerate(pre["slots"][c]):
            out[g] = oc[s]
    if _trace:
        kernel._last_exec_ns = res.exec_time_ns
    return out
